# revision 5
# baseline (speedup 1.0000x reference)
"""Trainium2 Bass kernel for NewHyperLinear (hypernetwork linear layer).

Reference computation:
    params  = noise @ hyper_W.T + hyper_b            # [B, IN*OUT + OUT]
    out     = einsum('bi,bio->bo', x, params_w) + params_b
    (+ same with prior_x / prior_W / prior_b)

Key algebraic restructuring (avoids materializing the 537MB params tensor):
    out[b,o] = S[b,o] + sum_n noise[b,n] * Q_n[b,o]
    Q_n[b,o] = sum_i x[b,i]*W4h[n,i,o] + sum_i px[b,i]*W4p[n,i,o]
    S[b,o]   = x @ Bh + px @ Bp + noise @ (Wbh+Wbp) + (hb_tail + pb_tail)

where W4 is hyper_W's weight part reshaped/transposed to [n, i, o] (host-side).
Q_n is a plain matmul per n (both W's accumulate into one PSUM bank); the
per-sample noise scale is a scalar_tensor_tensor with per-partition scalars
(batch on partitions).

Sharding over 8 cores: 4-way over OUT_F columns x 2-way over batch.
"""

import numpy as np

import concourse.bass as bass
import concourse.bacc as bacc
import concourse.mybir as mybir
import concourse.tile as tile
from concourse.bass_utils import run_bass_kernel_spmd

B, IN_F, OUT_F, NOISE = 512, 512, 512, 128
W_PART = IN_F * OUT_F  # 262144
PRIOR_SCALE = 1.0

OG, BG = 4, 2                 # o-groups x b-groups = 8 cores
O_SL = OUT_F // OG            # 128 output cols per core
B_SL = B // BG                # 256 batch rows per core
BC = B_SL // 128              # 2 batch chunks of 128 (PSUM partition limit)
IC = IN_F // 128              # 4 contraction chunks
NB = 4                        # noise dims per block (one PSUM bank = 4*128 fp32)
NBLK = NOISE // NB            # 32 blocks

F16 = mybir.dt.float16
F32 = mybir.dt.float32

_NC_CACHE = None


def _build_bass():
    nc = bacc.Bacc("TRN2", debug=False)

    # Per-core inputs. Layouts chosen so every dram->sbuf DMA is a single
    # transfer with >=1KB contiguous runs per partition.
    xt = nc.dram_tensor("xt", [128, IC, B_SL], F16, kind="ExternalInput")
    pxt = nc.dram_tensor("pxt", [128, IC, B_SL], F16, kind="ExternalInput")
    noiset = nc.dram_tensor("noiset", [NOISE, B_SL], F16, kind="ExternalInput")
    noise = nc.dram_tensor("noise", [128, BC, NOISE], F32, kind="ExternalInput")
    wh = nc.dram_tensor("wh", [128, IC, NOISE, O_SL], F16, kind="ExternalInput")
    wp = nc.dram_tensor("wp", [128, IC, NOISE, O_SL], F16, kind="ExternalInput")
    bmat = nc.dram_tensor("bmat", [128, IC, O_SL], F16, kind="ExternalInput")
    pbmat = nc.dram_tensor("pbmat", [128, IC, O_SL], F16, kind="ExternalInput")
    wb = nc.dram_tensor("wb", [NOISE, O_SL], F16, kind="ExternalInput")
    btail = nc.dram_tensor("btail", [1, O_SL], F16, kind="ExternalInput")
    out = nc.dram_tensor("out", [BC, 128, O_SL], F32, kind="ExternalOutput")

    ts = bass.ts

    with tile.TileContext(nc) as tc:
        with (
            tc.tile_pool(name="const", bufs=1) as cpool,
            tc.tile_pool(name="wpool", bufs=3) as wpool,
            tc.tile_pool(name="accp", bufs=1) as apool,
            tc.tile_pool(name="qpsum", bufs=5, space="PSUM") as qpool,
            tc.tile_pool(name="spsum", bufs=1, space="PSUM") as spool,
        ):
            xt_sb = cpool.tile([128, IC, B_SL], F16)
            nc.sync.dma_start(xt_sb[:], xt[:])
            pxt_sb = cpool.tile([128, IC, B_SL], F16)
            nc.sync.dma_start(pxt_sb[:], pxt[:])
            noiset_sb = cpool.tile([NOISE, B_SL], F16)
            nc.sync.dma_start(noiset_sb[:], noiset[:])
            noise_sb = cpool.tile([128, BC, NOISE], F32)
            nc.sync.dma_start(noise_sb[:], noise[:])
            bmat_sb = cpool.tile([128, IC, O_SL], F16)
            nc.sync.dma_start(bmat_sb[:], bmat[:])
            pbmat_sb = cpool.tile([128, IC, O_SL], F16)
            nc.sync.dma_start(pbmat_sb[:], pbmat[:])
            wb_sb = cpool.tile([NOISE, O_SL], F16)
            nc.sync.dma_start(wb_sb[:], wb[:])
            btail_sb = cpool.tile([1, O_SL], F16)
            nc.sync.dma_start(btail_sb[:], btail[:])
            ones_sb = cpool.tile([1, 128], F16)
            nc.vector.memset(ones_sb[:], 1.0)

            acc = apool.tile([128, BC, O_SL], F32)

            # Touch noise_sb on DVE once so the per-n scalar_tensor_tensor
            # ops don't need their own DMA-wait (the S2S2D2_STT instruction
            # has fewer sync-wait slots than other ops).
            scratch = cpool.tile([128, 1], F32)
            nc.vector.tensor_copy(scratch[:], noise_sb[:, 0, 0:1])

            # Secondary terms: S = x@Bh + px@Bp + noise@Wb + 1x1@btail
            for bc in range(BC):
                sp = spool.tile([128, O_SL], F32)
                for ic in range(IC):
                    nc.tensor.matmul(
                        sp[:], xt_sb[:, ic, ts(bc, 128)], bmat_sb[:, ic, :],
                        start=(ic == 0), stop=False,
                    )
                for ic in range(IC):
                    nc.tensor.matmul(
                        sp[:], pxt_sb[:, ic, ts(bc, 128)], pbmat_sb[:, ic, :],
                        start=False, stop=False,
                    )
                nc.tensor.matmul(
                    sp[:], noiset_sb[:, ts(bc, 128)], wb_sb[:],
                    start=False, stop=False,
                )
                nc.tensor.matmul(
                    sp[:], ones_sb[:], btail_sb[:], start=False, stop=True,
                )
                nc.vector.tensor_copy(acc[:, bc, :], sp[:])

            # Main loop: Q_n = x.T.T@W4h[n] + px.T.T@W4p[n]; acc += noise[:,n]*Q_n
            for blk in range(NBLK):
                n0 = blk * NB
                wh_sb = wpool.tile([128, IC, NB, O_SL], F16, tag="wh")
                nc.sync.dma_start(wh_sb[:], wh[:, :, n0:n0 + NB, :])
                wp_sb = wpool.tile([128, IC, NB, O_SL], F16, tag="wp")
                nc.sync.dma_start(wp_sb[:], wp[:, :, n0:n0 + NB, :])
                for bc in range(BC):
                    q = qpool.tile([128, NB, O_SL], F32)
                    for ic in range(IC):
                        nc.tensor.matmul(
                            q[:], xt_sb[:, ic, ts(bc, 128)], wh_sb[:, ic, :, :],
                            start=(ic == 0), stop=False,
                        )
                    for ic in range(IC):
                        nc.tensor.matmul(
                            q[:], pxt_sb[:, ic, ts(bc, 128)], wp_sb[:, ic, :, :],
                            start=False, stop=(ic == IC - 1),
                        )
                    for j in range(NB):
                        nc.vector.scalar_tensor_tensor(
                            acc[:, bc, :],
                            q[:, j, :],
                            noise_sb[:, bc, n0 + j:n0 + j + 1],
                            acc[:, bc, :],
                            mybir.AluOpType.mult,
                            mybir.AluOpType.add,
                        )

            for bc in range(BC):
                nc.sync.dma_start(out[bc], acc[:, bc, :])

    nc.compile()
    return nc


def get_nc():
    global _NC_CACHE
    if _NC_CACHE is None:
        _NC_CACHE = _build_bass()
    return _NC_CACHE


def _prep_in_maps(x, prior_x, hyper_noise, hyper_W, hyper_b, prior_W, prior_b):
    f16, f32 = np.float16, np.float32
    x = np.asarray(x, f32)
    prior_x = np.asarray(prior_x, f32)
    hyper_noise = np.asarray(hyper_noise, f32)
    hyper_W = np.asarray(hyper_W, f32)
    hyper_b = np.asarray(hyper_b, f32)
    prior_W = np.asarray(prior_W, f32)
    prior_b = np.asarray(prior_b, f32)
    if PRIOR_SCALE != 1.0:
        prior_W = prior_W * PRIOR_SCALE
        prior_b = prior_b * PRIOR_SCALE

    W3h = hyper_W[:W_PART].reshape(IN_F, OUT_F, NOISE)
    W3p = prior_W[:W_PART].reshape(IN_F, OUT_F, NOISE)
    wbT = (hyper_W[W_PART:] + prior_W[W_PART:]).T          # [NOISE, OUT_F]
    bmat_full = hyper_b[:W_PART].reshape(IN_F, OUT_F)
    pbmat_full = prior_b[:W_PART].reshape(IN_F, OUT_F)
    btail_full = (hyper_b[W_PART:] + prior_b[W_PART:]).reshape(1, OUT_F)

    # per o-group arrays
    wh_c, wp_c, bmat_c, pbmat_c, wb_c, btail_c = [], [], [], [], [], []
    for og in range(OG):
        osl = slice(og * O_SL, (og + 1) * O_SL)
        wh_c.append(
            W3h[:, osl, :].reshape(IC, 128, O_SL, NOISE)
            .transpose(1, 0, 3, 2).astype(f16)
        )
        wp_c.append(
            W3p[:, osl, :].reshape(IC, 128, O_SL, NOISE)
            .transpose(1, 0, 3, 2).astype(f16)
        )
        bmat_c.append(
            bmat_full[:, osl].reshape(IC, 128, O_SL).transpose(1, 0, 2).astype(f16)
        )
        pbmat_c.append(
            pbmat_full[:, osl].reshape(IC, 128, O_SL).transpose(1, 0, 2).astype(f16)
        )
        wb_c.append(np.ascontiguousarray(wbT[:, osl]).astype(f16))
        btail_c.append(np.ascontiguousarray(btail_full[:, osl]).astype(f16))

    # per b-group arrays
    xt_c, pxt_c, noiset_c, noise_c = [], [], [], []
    for bg in range(BG):
        bsl = slice(bg * B_SL, (bg + 1) * B_SL)
        xt_c.append(
            x[bsl].T.reshape(IC, 128, B_SL).transpose(1, 0, 2).astype(f16)
        )
        pxt_c.append(
            prior_x[bsl].T.reshape(IC, 128, B_SL).transpose(1, 0, 2).astype(f16)
        )
        noiset_c.append(np.ascontiguousarray(hyper_noise[bsl].T).astype(f16))
        noise_c.append(
            hyper_noise[bsl].reshape(BC, 128, NOISE).transpose(1, 0, 2)
            .astype(f32)
        )

    in_maps = []
    for cid in range(OG * BG):
        og, bg = cid % OG, cid // OG
        in_maps.append({
            "xt": xt_c[bg],
            "pxt": pxt_c[bg],
            "noiset": noiset_c[bg],
            "noise": noise_c[bg],
            "wh": wh_c[og],
            "wp": wp_c[og],
            "bmat": bmat_c[og],
            "pbmat": pbmat_c[og],
            "wb": wb_c[og],
            "btail": btail_c[og],
        })
    return in_maps


def run(trace=False, **inputs):
    """Run the kernel; returns (full_output, BassKernelResults)."""
    nc = get_nc()
    in_maps = _prep_in_maps(**inputs)
    res = run_bass_kernel_spmd(
        nc, in_maps, core_ids=list(range(OG * BG)), trace=trace,
    )
    full = np.empty((B, OUT_F), np.float32)
    for cid in range(OG * BG):
        og, bg = cid % OG, cid // OG
        shard = res.results[cid]["out"].reshape(B_SL, O_SL)
        full[bg * B_SL:(bg + 1) * B_SL, og * O_SL:(og + 1) * O_SL] = shard
    return full, res


def kernel(**inputs):
    return run(trace=False, **inputs)[0]
